# revision 5
# baseline (speedup 1.0000x reference)
"""Trainium2 Bass kernel for CodeRecursiveNeuralNetworks (tree-RNN over complete
binary trees, heap layout).

Math (per tree, heap order: node i has parent (i-1)//2, level d = [2^d-1, 2^{d+1}-1)):
    x = E[node_type];  h_leaf = tanh(x_leaf)
    for d = 8..0:  h_d = tanh(x_d + (h_{d+1,even} + h_{d+1,odd}) @ Wh + bh)
    logits = h_root @ Wc + bc;  out = log_softmax(logits)

Strategy (8 cores, data-parallel over trees; 32 trees/core, no collectives):
  - Everything in "transposed" layout [H=128 partitions, nodes free].
  - Split-ordered levels: the host permutes each level's columns so that the
    even/odd children of any 1024-aligned parent group occupy two contiguous
    halves of one 2048-col child chunk. All pair reductions become contiguous
    slices: no strided access anywhere; the pair-sum is folded into the PE as
    two contiguous Wh matmuls (no DVE hop in the recursion).
  - Embedding lookups are one-hot matmuls (VOCAB=100 <= 128); host re-encodes
    node_type as fp8 one-hot columns (index re-encoding only).
  - Leaf level folded away: G = tanh(E) @ Wh on device; level 8 computes
    psum8 = E^T @ X8 + G^T @ C8 where C8[:,j] = onehot(a_j)+onehot(b_j).
  - Level-8 chunks laddered (512,512,1024,2048x3) so the tanh stream starts
    early; the ACT engine is the roofline (~16.4us of tanh) and runs nothing
    but activations (all DMA triggers live on Sync/GpSimd/Vector).
  - tanh+bias fused on ScalarE reading PSUM directly; h stored fp16.
  - Tiny PE warm-up (2 junk matmuls) + small fillers in the serial tail to
    hold the PE p-state; log_softmax on device; per-core output [32,6] fp32.
"""

import numpy as np
import ml_dtypes

B = 256
M = 1023
H = 128
V = 100
NCLS = 6
CORES = 8
TPC = B // CORES          # trees per core (32)

# per-core level sizes: level d has TPC * 2^d columns
LVL_N = {d: TPC * (1 << d) for d in range(10)}

# ---- split-order permutations (core-independent, level-local indexing) ----
# ord_d[j] = original (tree-major) level-local index of stored column j.
# Children of a 1024-aligned stored parent group land as [evens | odds] in one
# contiguous 2048-col child chunk.
_ORD = {0: np.arange(TPC, dtype=np.int64)}
for _d in range(1, 9):
    prev = _ORD[_d - 1]
    grp = min(prev.size, 1024)
    parts = []
    for g in range(0, prev.size, grp):
        p = prev[g:g + grp]
        parts.append(2 * p)
        parts.append(2 * p + 1)
    _ORD[_d] = np.concatenate(parts)

# ---- oh dram layout [V, TOTAL] ----
# 4 level-8 bands of 4096: band k = [x8 cols 2048k..+2048 | c8 same cols]
# then x7 (4096), then x6..x0 packed (4064)
OFF_B8 = [4096 * k for k in range(4)]
OFF_X7 = 16384
OFF_REST = 20480
REST_OFF = {}            # level -> offset within the rest band
_o = 0
for _d in range(6, -1, -1):
    REST_OFF[_d] = _o
    _o += LVL_N[_d]
REST_N = _o              # 4064
TOTAL_COLS = OFF_REST + REST_N

_PROGRAM = None


def _build_program():
    import concourse.bacc as bacc
    import concourse.tile as tile
    import concourse.mybir as mybir
    from concourse.masks import make_identity

    dt = mybir.dt
    AF = mybir.ActivationFunctionType
    ALU = mybir.AluOpType
    AX = mybir.AxisListType

    nc = bacc.Bacc("TRN2", target_bir_lowering=False, debug=False)

    oh_d = nc.dram_tensor("oh", [V, TOTAL_COLS], dt.float8e4, kind="ExternalInput")
    ebf_d = nc.dram_tensor("e_bf", [V, H], dt.float16, kind="ExternalInput")
    et_d = nc.dram_tensor("e_t", [H, V], dt.float32, kind="ExternalInput")
    whb_d = nc.dram_tensor("wh_b", [H, H], dt.float16, kind="ExternalInput")
    bh_d = nc.dram_tensor("bh", [H, 1], dt.float32, kind="ExternalInput")
    wc_d = nc.dram_tensor("wc", [H, NCLS], dt.float32, kind="ExternalInput")
    bc_d = nc.dram_tensor("bc", [NCLS, 1], dt.float32, kind="ExternalInput")
    out_d = nc.dram_tensor("out", [TPC, NCLS], dt.float32, kind="ExternalOutput")

    with tile.TileContext(nc) as tc:
        with (
            tc.tile_pool(name="const", bufs=1) as cpool,
            tc.tile_pool(name="bandp", bufs=1) as bandpool,
            tc.tile_pool(name="hp", bufs=1) as hpool,
            tc.tile_pool(name="psp", bufs=2, space="PSUM") as pspool,
            tc.tile_pool(name="smallp", bufs=1) as smpool,
        ):
            # ---- junk for PE warm-up / fillers: memset first on gpsimd ----
            junk = cpool.tile([H, 512], dt.bfloat16, tag="junk")
            nc.gpsimd.memset(junk[:], 0)
            # dummy tanh: pulls the ACT table load off the critical path
            dummy_t = smpool.tile([H, 1], dt.float16, tag="dummy")
            nc.scalar.activation(dummy_t[:], junk[:, :1], AF.Tanh)

            # ---- constants on the gpsimd queue ----
            e_bf = cpool.tile([V, H], dt.float16, tag="e_bf")
            nc.gpsimd.dma_start(out=e_bf[:], in_=ebf_d[:])
            whb = cpool.tile([H, H], dt.float16, tag="whb")
            nc.gpsimd.dma_start(out=whb[:], in_=whb_d[:])
            et = cpool.tile([H, V], dt.float32, tag="et")
            nc.gpsimd.dma_start(out=et[:], in_=et_d[:])
            bh_t = cpool.tile([H, 1], dt.float32, tag="bh")
            nc.gpsimd.dma_start(out=bh_t[:], in_=bh_d[:])
            wc_t = cpool.tile([H, NCLS], dt.float32, tag="wc")
            nc.gpsimd.dma_start(out=wc_t[:], in_=wc_d[:])
            bc_t = cpool.tile([NCLS, 1], dt.float32, tag="bc")
            nc.gpsimd.dma_start(out=bc_t[:], in_=bc_d[:])

            # ---- band tiles + DMA triggers (Sync / GpSimd / Vector) ----
            b8 = [bandpool.tile([V, 4096], dt.float8e4, tag=f"b8_{k}",
                                name=f"b8_{k}") for k in range(4)]
            bx7 = bandpool.tile([V, 4096], dt.float8e4, tag="bx7")
            brest = bandpool.tile([V, REST_N], dt.float8e4, tag="brest")
            # x8 half of band0 first (gates the very first matmul), then c8
            nc.sync.dma_start(out=b8[0][:, :2048], in_=oh_d[:, 0:2048])
            nc.sync.dma_start(out=b8[0][:, 2048:], in_=oh_d[:, 2048:4096])
            nc.sync.dma_start(out=b8[1][:], in_=oh_d[:, 4096:8192])
            nc.sync.dma_start(out=b8[2][:], in_=oh_d[:, 8192:12288])
            nc.sync.dma_start(out=b8[3][:], in_=oh_d[:, 12288:16384])
            nc.sync.dma_start(out=bx7[:], in_=oh_d[:, OFF_X7:OFF_X7 + 4096])

            def x_slice(d, col, w):
                """one-hot slice for level d, stored cols [col, col+w)."""
                if d == 8:
                    k, off = divmod(col, 2048)
                    return b8[k][:, off:off + w]
                if d == 7:
                    return bx7[:, col:col + w]
                off = REST_OFF[d] + col
                return brest[:, off:off + w]

            def c_slice(col, w):
                k, off = divmod(col, 2048)
                return b8[k][:, 2048 + off:2048 + off + w]

            # ---- PE warm-up (2 junk matmuls into a pool psum tile) ----
            warm_ps = pspool.tile([H, 512], dt.float32, tag="ps", name="warm_ps")
            for _ in range(2):
                nc.tensor.matmul(warm_ps[:], lhsT=junk[:, :H], rhs=junk[:],
                                 start=True, stop=True)

            def filler(n=256):
                fps = pspool.tile([H, n], dt.float32, tag="ps", name="filler")
                nc.tensor.matmul(fps[:], lhsT=junk[:, :H], rhs=junk[:, :n],
                                 start=True, stop=True)

            # ---- G = tanh(E) @ Wh ----
            tanh_et = cpool.tile([H, V], dt.float16, tag="tanh_et")
            nc.scalar.activation(tanh_et[:], et[:], AF.Tanh)
            g_ps = pspool.tile([V, H], dt.float32, tag="ps", name="g_ps")
            nc.tensor.matmul(g_ps[:], lhsT=tanh_et[:], rhs=whb[:],
                             start=True, stop=True)
            g_sb = cpool.tile([V, H], dt.float16, tag="g_sb")
            nc.vector.tensor_copy(g_sb[:], g_ps[:])
            wc16 = cpool.tile([H, NCLS], dt.float16, tag="wc16")
            nc.vector.tensor_copy(wc16[:], wc_t[:])
            # rest band last on the gpsimd queue (needed only from level 6 on)
            nc.gpsimd.dma_start(out=brest[:], in_=oh_d[:, OFF_REST:TOTAL_COLS])

            # ---- h tiles ----
            # level 8: 4 tiles of 2048; levels 7,6: tiles of 2048; 5..0 single
            h8 = [hpool.tile([H, 2048], dt.float16, tag=f"h8_{k}",
                             name=f"h8_{k}") for k in range(4)]
            h7 = [hpool.tile([H, 2048], dt.float16, tag=f"h7_{k}",
                             name=f"h7_{k}") for k in range(2)]
            h6 = hpool.tile([H, 2048], dt.float16, tag="h6")
            hsm = {d: hpool.tile([H, LVL_N[d]], dt.float16, tag=f"h{d}",
                                 name=f"h{d}") for d in range(6)}

            def h_tile(d, col):
                """(tile, offset) holding stored column `col` of level d."""
                if d == 8:
                    k, off = divmod(col, 2048)
                    return h8[k], off
                if d == 7:
                    k, off = divmod(col, 2048)
                    return h7[k], off
                if d == 6:
                    return h6, col
                return hsm[d], col

            # ---- level 8: laddered chunks ----
            l8_chunks = [(0, 512), (512, 512), (1024, 1024),
                         (2048, 2048), (4096, 2048), (6144, 2048)]
            for c0, cn in l8_chunks:
                ps = pspool.tile([H, cn], dt.float32, tag="ps", name=f"ps8_{c0}")
                for s in range(0, cn, 512):
                    nc.tensor.matmul(ps[:, s:s + 512], lhsT=e_bf[:],
                                     rhs=x_slice(8, c0 + s, 512),
                                     start=True, stop=False)
                for s in range(0, cn, 512):
                    nc.tensor.matmul(ps[:, s:s + 512], lhsT=g_sb[:],
                                     rhs=c_slice(c0 + s, 512),
                                     start=False, stop=True)
                ht, off = h_tile(8, c0)
                nc.scalar.activation(ht[:, off:off + cn], ps[:], AF.Tanh,
                                     bias=bh_t[:])

            # ---- levels 7..0: E one-hot + PE pair-sum (contiguous halves) ----
            # level-d chunking: 7 -> 2x2048, 6 -> 2x1024, else single chunk
            def level_chunks(d):
                n = LVL_N[d]
                if d == 7:
                    return [(0, 2048), (2048, 2048)]
                if d == 6:
                    return [(0, 1024), (1024, 1024)]
                return [(0, n)]

            for d in range(7, -1, -1):
                emitted = []
                for c0, cn in level_chunks(d):
                    ps = pspool.tile([H, cn], dt.float32, tag="ps",
                                     name=f"ps{d}_{c0}")
                    for s in range(0, cn, 512):
                        w = min(512, cn - s)
                        nc.tensor.matmul(ps[:, s:s + w], lhsT=e_bf[:],
                                         rhs=x_slice(d, c0 + s, w),
                                         start=True, stop=False)
                    # children of stored col j live at h_{d+1}[grp_base + r]
                    # (even) and [grp_base + GRP + r] (odd), GRP = 1024 for
                    # d+1 >= 6 (2048-col chunks), else half the child level.
                    grp = 1024 if LVL_N[d + 1] >= 2048 else LVL_N[d + 1] // 2
                    for s in range(0, cn, 512):
                        w = min(512, cn - s)
                        j = c0 + s
                        g, r = divmod(j, grp)
                        ct, coff = h_tile(d + 1, g * 2 * grp)
                        nc.tensor.matmul(ps[:, s:s + w], lhsT=whb[:],
                                         rhs=ct[:, coff + r:coff + r + w],
                                         start=False, stop=False)
                        nc.tensor.matmul(ps[:, s:s + w], lhsT=whb[:],
                                         rhs=ct[:, coff + grp + r:coff + grp + r + w],
                                         start=False, stop=True)
                    emitted.append((ps, c0, cn))
                for ps, c0, cn in emitted:
                    ht, off = h_tile(d, c0)
                    nc.scalar.activation(ht[:, off:off + cn], ps[:, :cn],
                                         AF.Tanh, bias=bh_t[:])
                if d <= 4:
                    filler()

            # ---- logits + log_softmax ----
            id6 = cpool.tile([NCLS, NCLS], dt.float32, tag="id6")
            make_identity(nc, id6[:])
            h0 = hsm[0]                               # [H, TPC]
            lg_ps = pspool.tile([NCLS, TPC], dt.float32, tag="ps", name="lg_ps")
            nc.tensor.matmul(lg_ps[:], lhsT=wc16[:], rhs=h0[:],
                             start=True, stop=True)
            lgT = smpool.tile([NCLS, TPC], dt.float32, tag="lgT")
            nc.vector.tensor_tensor(out=lgT[:], in0=lg_ps[:],
                                    in1=bc_t[:].to_broadcast([NCLS, TPC]),
                                    op=ALU.add)
            tr_ps = pspool.tile([TPC, NCLS], dt.float32, tag="ps", name="tr_ps")
            nc.tensor.transpose(tr_ps[:], lgT[:], id6[:])
            lg = smpool.tile([TPC, NCLS], dt.float32, tag="lg")
            nc.vector.tensor_copy(lg[:], tr_ps[:])
            ex = smpool.tile([TPC, NCLS], dt.float32, tag="ex")
            nc.scalar.activation(ex[:], lg[:], AF.Exp)
            s = smpool.tile([TPC, 1], dt.float32, tag="s")
            nc.vector.tensor_reduce(s[:], ex[:], axis=AX.X, op=ALU.add)
            lns = smpool.tile([TPC, 1], dt.float32, tag="lns")
            nc.scalar.activation(lns[:], s[:], AF.Ln)
            res = smpool.tile([TPC, NCLS], dt.float32, tag="res")
            nc.vector.tensor_scalar(out=res[:], in0=lg[:], scalar1=lns[:],
                                    scalar2=None, op0=ALU.subtract)
            nc.sync.dma_start(out=out_d[:], in_=res[:])

    nc.compile()
    return nc, "out"


def _get_program():
    global _PROGRAM
    if _PROGRAM is None:
        _PROGRAM = _build_program()
    return _PROGRAM


def _host_inputs(node_type, E, Wh, bh, Wc, bc):
    """Build per-core input maps (host side: sharding + index re-encoding)."""
    nt = np.asarray(node_type).astype(np.int64).reshape(B, M)
    E = np.asarray(E, dtype=np.float32)
    Wh = np.asarray(Wh, dtype=np.float32)
    bh = np.asarray(bh, dtype=np.float32)
    Wc = np.asarray(Wc, dtype=np.float32)
    bc = np.asarray(bc, dtype=np.float32)

    shared = {
        "e_bf": E.astype(np.float16),
        "e_t": np.ascontiguousarray(E.T),
        "wh_b": Wh.astype(np.float16),
        "bh": bh.reshape(H, 1),
        "wc": Wc,
        "bc": bc.reshape(NCLS, 1),
    }

    ncols = {d: np.arange(LVL_N[d]) for d in range(9)}
    in_maps = []
    for c in range(CORES):
        ntc = nt[c * TPC:(c + 1) * TPC]                  # [32, 1023]
        ohf = np.zeros((V, TOTAL_COLS), dtype=np.float32)
        # level types in tree-major local order, then permute to stored order
        lvl = {d: ntc[:, (1 << d) - 1:(2 << d) - 1].reshape(-1) for d in range(9)}
        leaves = ntc[:, 511:1023].reshape(-1)            # [16384]
        # level 8 bands: x8 + c8 (summed leaf-children one-hots)
        t8 = lvl[8][_ORD[8]]
        a = leaves[2 * _ORD[8]]
        b = leaves[2 * _ORD[8] + 1]
        for k in range(4):
            cols = np.arange(2048)
            ohf[t8[2048 * k + cols], OFF_B8[k] + cols] = 1.0
            np.add.at(ohf, (a[2048 * k + cols], OFF_B8[k] + 2048 + cols), 1.0)
            np.add.at(ohf, (b[2048 * k + cols], OFF_B8[k] + 2048 + cols), 1.0)
        # x7 band
        ohf[lvl[7][_ORD[7]], OFF_X7 + ncols[7]] = 1.0
        # x6..x0
        for d in range(6, -1, -1):
            ohf[lvl[d][_ORD[d]], OFF_REST + REST_OFF[d] + ncols[d]] = 1.0
        in_maps.append({"oh": ohf.astype(ml_dtypes.float8_e4m3), **shared})
    return in_maps


def kernel(node_type, parent_idx, depth, root_idx, E, Wh, bh, Wc, bc,
           _trace=False, _sim=False):
    from concourse.bass_utils import run_bass_kernel_spmd

    nc, out_name = _get_program()
    in_maps = _host_inputs(node_type, E, Wh, bh, Wc, bc)

    if _sim:
        from concourse.bass_interp import CoreSim
        outs = []
        for m in in_maps[:_sim if isinstance(_sim, int) and _sim > 1 else CORES]:
            sim = CoreSim(nc, trace=False)
            for k, v in m.items():
                sim.tensor(k)[:] = v
            sim.simulate(check_with_hw=False)
            outs.append(np.array(sim.tensor(out_name)))
        return np.concatenate(outs, axis=0).astype(np.float32)

    results = run_bass_kernel_spmd(
        nc, in_maps, core_ids=list(range(CORES)), trace=_trace,
    )
    out = np.concatenate([r[out_name] for r in results.results], axis=0)
    out = np.ascontiguousarray(out).astype(np.float32)
    if _trace:
        return out, results
    return out


# revision 9
# speedup vs baseline: 1.0358x; 1.0358x over previous
"""Trainium2 Bass kernel for CodeRecursiveNeuralNetworks (tree-RNN over complete
binary trees, heap layout).

Math (per tree, heap order: node i has parent (i-1)//2, level d = [2^d-1, 2^{d+1}-1)):
    x = E[node_type];  h_leaf = tanh(x_leaf)
    for d = 8..0:  h_d = tanh(x_d + (h_{d+1,even} + h_{d+1,odd}) @ Wh + bh)
    logits = h_root @ Wc + bc;  out = log_softmax(logits)

Strategy (8 cores, data-parallel over trees; 32 trees/core, no collectives):
  - Everything in "transposed" layout [H=128 partitions, nodes free].
  - Split-ordered levels: the host permutes each level's columns so that the
    even/odd children of any 1024-aligned parent group occupy two contiguous
    halves of one 2048-col child chunk. All pair reductions become contiguous
    slices: no strided access anywhere; the pair-sum is folded into the PE as
    two contiguous Wh matmuls (no DVE hop in the recursion).
  - Embedding lookups are one-hot matmuls (VOCAB=100 <= 128); host re-encodes
    node_type as fp8 one-hot columns (index re-encoding only).
  - Leaf level folded away: G = tanh(E) @ Wh on device; level 8 computes
    psum8 = E^T @ X8 + G^T @ C8 where C8[:,j] = onehot(a_j)+onehot(b_j).
  - Level-8 chunks laddered (512,512,1024,2048x3) so the tanh stream starts
    early; the ACT engine is the roofline (~16.4us of tanh) and runs nothing
    but activations (all DMA triggers live on Sync/GpSimd/Vector).
  - tanh+bias fused on ScalarE reading PSUM directly; h stored fp16.
  - Tiny PE warm-up (2 junk matmuls) + small fillers in the serial tail to
    hold the PE p-state; log_softmax on device; per-core output [32,6] fp32.
"""

import numpy as np
import ml_dtypes

B = 256
M = 1023
H = 128
V = 100
NCLS = 6
CORES = 8
TPC = B // CORES          # trees per core (32)

# per-core level sizes: level d has TPC * 2^d columns
LVL_N = {d: TPC * (1 << d) for d in range(10)}

# ---- split-order permutations (core-independent, level-local indexing) ----
# ord_d[j] = original (tree-major) level-local index of stored column j.
# Children of a 1024-aligned stored parent group land as [evens | odds] in one
# contiguous 2048-col child chunk.
_ORD = {0: np.arange(TPC, dtype=np.int64)}
for _d in range(1, 9):
    prev = _ORD[_d - 1]
    grp = min(prev.size, 1024)
    parts = []
    for g in range(0, prev.size, grp):
        p = prev[g:g + grp]
        parts.append(2 * p)
        parts.append(2 * p + 1)
    _ORD[_d] = np.concatenate(parts)

# ---- oh dram layout [V, TOTAL] ----
# 4 level-8 bands of 4096: band k = [x8 cols 2048k..+2048 | c8 same cols]
# then x7 (4096), then x6..x0 packed (4064)
OFF_B8 = [4096 * k for k in range(4)]
OFF_X7 = 16384
OFF_REST = 20480
REST_OFF = {}            # level -> offset within the rest band
_o = 0
for _d in range(6, -1, -1):
    REST_OFF[_d] = _o
    _o += LVL_N[_d]
REST_N = _o              # 4064
TOTAL_COLS = OFF_REST + REST_N

_PROGRAM = None


def _build_program():
    import concourse.bacc as bacc
    import concourse.tile as tile
    import concourse.mybir as mybir
    from concourse.masks import make_identity

    dt = mybir.dt
    AF = mybir.ActivationFunctionType
    ALU = mybir.AluOpType
    AX = mybir.AxisListType

    nc = bacc.Bacc("TRN2", target_bir_lowering=False, debug=False)

    oh_d = nc.dram_tensor("oh", [V, TOTAL_COLS], dt.float8e4, kind="ExternalInput")
    ebf_d = nc.dram_tensor("e_bf", [V, H], dt.float16, kind="ExternalInput")
    et_d = nc.dram_tensor("e_t", [H, V], dt.float32, kind="ExternalInput")
    whb_d = nc.dram_tensor("wh_b", [H, H], dt.float16, kind="ExternalInput")
    bh_d = nc.dram_tensor("bh", [H, 1], dt.float32, kind="ExternalInput")
    wc_d = nc.dram_tensor("wc", [H, NCLS], dt.float32, kind="ExternalInput")
    bc_d = nc.dram_tensor("bc", [NCLS, 1], dt.float32, kind="ExternalInput")
    out_d = nc.dram_tensor("out", [TPC, NCLS], dt.float32, kind="ExternalOutput")

    with tile.TileContext(nc) as tc:
        with (
            tc.tile_pool(name="const", bufs=1) as cpool,
            tc.tile_pool(name="bandp", bufs=1) as bandpool,
            tc.tile_pool(name="hp", bufs=1) as hpool,
            tc.tile_pool(name="psp", bufs=2, space="PSUM") as pspool,
            tc.tile_pool(name="smallp", bufs=1) as smpool,
        ):
            # ---- junk for PE warm-up / fillers: memset first on vector ----
            junk = cpool.tile([H, 512], dt.bfloat16, tag="junk")
            nc.vector.memset(junk[:], 0)
            # dummy tanh: pulls the ACT table load off the critical path
            dummy_t = smpool.tile([H, 1], dt.float16, tag="dummy")
            nc.scalar.activation(dummy_t[:], junk[:, :1], AF.Tanh)

            # ---- constants on the gpsimd queue (G-chain + bias first) ----
            et = cpool.tile([H, V], dt.float32, tag="et")
            nc.gpsimd.dma_start(out=et[:], in_=et_d[:])
            whb = cpool.tile([H, H], dt.float16, tag="whb")
            nc.gpsimd.dma_start(out=whb[:], in_=whb_d[:])
            e_bf = cpool.tile([V, H], dt.float16, tag="e_bf")
            nc.gpsimd.dma_start(out=e_bf[:], in_=ebf_d[:])
            bh_t = cpool.tile([H, 1], dt.float32, tag="bh")
            nc.gpsimd.dma_start(out=bh_t[:], in_=bh_d[:])
            wc_t = cpool.tile([H, NCLS], dt.float32, tag="wc")
            nc.gpsimd.dma_start(out=wc_t[:], in_=wc_d[:])
            bc_t = cpool.tile([NCLS, 1], dt.float32, tag="bc")
            nc.gpsimd.dma_start(out=bc_t[:], in_=bc_d[:])

            # ---- band tiles + DMA triggers (Sync, in consumption order) ----
            b8 = [bandpool.tile([V, 4096], dt.float8e4, tag=f"b8_{k}",
                                name=f"b8_{k}") for k in range(4)]
            bx7 = bandpool.tile([V, 4096], dt.float8e4, tag="bx7")
            brest = bandpool.tile([V, REST_N], dt.float8e4, tag="brest")
            # fine-grained x/c interleave for band0 so the ladder starts early
            nc.sync.dma_start(out=b8[0][:, :1024], in_=oh_d[:, 0:1024])
            nc.sync.dma_start(out=b8[0][:, 2048:3072], in_=oh_d[:, 2048:3072])
            nc.sync.dma_start(out=b8[0][:, 1024:2048], in_=oh_d[:, 1024:2048])
            nc.sync.dma_start(out=b8[0][:, 3072:4096], in_=oh_d[:, 3072:4096])
            nc.sync.dma_start(out=b8[1][:], in_=oh_d[:, 4096:8192])
            nc.sync.dma_start(out=b8[2][:], in_=oh_d[:, 8192:12288])
            nc.sync.dma_start(out=b8[3][:], in_=oh_d[:, 12288:16384])
            nc.sync.dma_start(out=bx7[:], in_=oh_d[:, OFF_X7:OFF_X7 + 4096])

            def x_slice(d, col, w):
                """one-hot slice for level d, stored cols [col, col+w)."""
                if d == 8:
                    k, off = divmod(col, 2048)
                    return b8[k][:, off:off + w]
                if d == 7:
                    return bx7[:, col:col + w]
                off = REST_OFF[d] + col
                return brest[:, off:off + w]

            def c_slice(col, w):
                k, off = divmod(col, 2048)
                return b8[k][:, 2048 + off:2048 + off + w]

            # ---- PE warm-up (junk matmuls bridging the first-DMA wait) ----
            warm_ps = pspool.tile([H, 512], dt.float32, tag="ps", name="warm_ps")
            for _ in range(4):
                nc.tensor.matmul(warm_ps[:], lhsT=junk[:, :H], rhs=junk[:],
                                 start=True, stop=True)

            def filler(n=256):
                fps = pspool.tile([H, n], dt.float32, tag="ps", name="filler")
                nc.tensor.matmul(fps[:], lhsT=junk[:, :H], rhs=junk[:, :n],
                                 start=True, stop=True)

            # ---- G = tanh(E) @ Wh ----
            tanh_et = cpool.tile([H, V], dt.float16, tag="tanh_et")
            nc.scalar.activation(tanh_et[:], et[:], AF.Tanh)
            g_ps = pspool.tile([V, H], dt.float32, tag="ps", name="g_ps")
            nc.tensor.matmul(g_ps[:], lhsT=tanh_et[:], rhs=whb[:],
                             start=True, stop=True)
            g_sb = cpool.tile([V, H], dt.float16, tag="g_sb")
            nc.vector.tensor_copy(g_sb[:], g_ps[:])
            wc16 = cpool.tile([H, NCLS], dt.float16, tag="wc16")
            nc.vector.tensor_copy(wc16[:], wc_t[:])
            # rest band last on the gpsimd queue (needed only from level 6 on)
            nc.gpsimd.dma_start(out=brest[:], in_=oh_d[:, OFF_REST:TOTAL_COLS])

            # ---- h tiles ----
            # level 8: 4 tiles of 2048; levels 7,6: tiles of 2048; 5..0 single
            h8 = [hpool.tile([H, 2048], dt.float16, tag=f"h8_{k}",
                             name=f"h8_{k}") for k in range(4)]
            h7 = [hpool.tile([H, 2048], dt.float16, tag=f"h7_{k}",
                             name=f"h7_{k}") for k in range(2)]
            h6 = hpool.tile([H, 2048], dt.float16, tag="h6")
            hsm = {d: hpool.tile([H, LVL_N[d]], dt.float16, tag=f"h{d}",
                                 name=f"h{d}") for d in range(6)}

            def h_tile(d, col):
                """(tile, offset) holding stored column `col` of level d."""
                if d == 8:
                    k, off = divmod(col, 2048)
                    return h8[k], off
                if d == 7:
                    k, off = divmod(col, 2048)
                    return h7[k], off
                if d == 6:
                    return h6, col
                return hsm[d], col

            # ---- level 8: laddered chunks ----
            l8_chunks = [(0, 512), (512, 512), (1024, 1024),
                         (2048, 2048), (4096, 2048), (6144, 2048)]
            for c0, cn in l8_chunks:
                ps = pspool.tile([H, cn], dt.float32, tag="ps", name=f"ps8_{c0}")
                for s in range(0, cn, 512):
                    nc.tensor.matmul(ps[:, s:s + 512], lhsT=e_bf[:],
                                     rhs=x_slice(8, c0 + s, 512),
                                     start=True, stop=False)
                for s in range(0, cn, 512):
                    nc.tensor.matmul(ps[:, s:s + 512], lhsT=g_sb[:],
                                     rhs=c_slice(c0 + s, 512),
                                     start=False, stop=True)
                ht, off = h_tile(8, c0)
                nc.scalar.activation(ht[:, off:off + cn], ps[:], AF.Tanh,
                                     bias=bh_t[:])

            # ---- levels 7..0: E one-hot + PE pair-sum (contiguous halves) ----
            # level-d chunking: 7 -> 2x2048, 6 -> 2x1024, else single chunk
            def level_chunks(d):
                n = LVL_N[d]
                if d == 7:
                    return [(0, 2048), (2048, 2048)]
                if d == 6:
                    return [(0, 1024), (1024, 1024)]
                return [(0, n)]

            for d in range(7, -1, -1):
                emitted = []
                for c0, cn in level_chunks(d):
                    ps = pspool.tile([H, cn], dt.float32, tag="ps",
                                     name=f"ps{d}_{c0}")
                    for s in range(0, cn, 512):
                        w = min(512, cn - s)
                        nc.tensor.matmul(ps[:, s:s + w], lhsT=e_bf[:],
                                         rhs=x_slice(d, c0 + s, w),
                                         start=True, stop=False)
                    # children of stored col j live at h_{d+1}[grp_base + r]
                    # (even) and [grp_base + GRP + r] (odd), GRP = 1024 for
                    # d+1 >= 6 (2048-col chunks), else half the child level.
                    grp = 1024 if LVL_N[d + 1] >= 2048 else LVL_N[d + 1] // 2
                    for s in range(0, cn, 512):
                        w = min(512, cn - s)
                        j = c0 + s
                        g, r = divmod(j, grp)
                        ct, coff = h_tile(d + 1, g * 2 * grp)
                        nc.tensor.matmul(ps[:, s:s + w], lhsT=whb[:],
                                         rhs=ct[:, coff + r:coff + r + w],
                                         start=False, stop=False)
                        nc.tensor.matmul(ps[:, s:s + w], lhsT=whb[:],
                                         rhs=ct[:, coff + grp + r:coff + grp + r + w],
                                         start=False, stop=True)
                    emitted.append((ps, c0, cn))
                for ps, c0, cn in emitted:
                    ht, off = h_tile(d, c0)
                    nc.scalar.activation(ht[:, off:off + cn], ps[:, :cn],
                                         AF.Tanh, bias=bh_t[:])
                if d <= 4:
                    filler()

            # ---- logits + log_softmax ----
            id6 = cpool.tile([NCLS, NCLS], dt.float32, tag="id6")
            make_identity(nc, id6[:])
            # dummy ln right after the last tanh: prefetches the ln+exp act
            # table while the logits matmul/transpose chain runs
            dummy_ln = smpool.tile([1, 1], dt.float32, tag="dummy_ln")
            nc.scalar.activation(dummy_ln[:], id6[:1, :1], AF.Ln)
            h0 = hsm[0]                               # [H, TPC]
            lg_ps = pspool.tile([NCLS, TPC], dt.float32, tag="ps", name="lg_ps")
            nc.tensor.matmul(lg_ps[:], lhsT=wc16[:], rhs=h0[:],
                             start=True, stop=True)
            lgT = smpool.tile([NCLS, TPC], dt.float32, tag="lgT")
            nc.vector.tensor_tensor(out=lgT[:], in0=lg_ps[:],
                                    in1=bc_t[:].to_broadcast([NCLS, TPC]),
                                    op=ALU.add)
            tr_ps = pspool.tile([TPC, NCLS], dt.float32, tag="ps", name="tr_ps")
            nc.tensor.transpose(tr_ps[:], lgT[:], id6[:])
            lg = smpool.tile([TPC, NCLS], dt.float32, tag="lg")
            nc.vector.tensor_copy(lg[:], tr_ps[:])
            ex = smpool.tile([TPC, NCLS], dt.float32, tag="ex")
            nc.scalar.activation(ex[:], lg[:], AF.Exp)
            s = smpool.tile([TPC, 1], dt.float32, tag="s")
            nc.vector.tensor_reduce(s[:], ex[:], axis=AX.X, op=ALU.add)
            lns = smpool.tile([TPC, 1], dt.float32, tag="lns")
            nc.scalar.activation(lns[:], s[:], AF.Ln)
            res = smpool.tile([TPC, NCLS], dt.float32, tag="res")
            nc.vector.tensor_scalar(out=res[:], in0=lg[:], scalar1=lns[:],
                                    scalar2=None, op0=ALU.subtract)
            nc.sync.dma_start(out=out_d[:], in_=res[:])

    nc.compile()
    return nc, "out"


def _get_program():
    global _PROGRAM
    if _PROGRAM is None:
        _PROGRAM = _build_program()
    return _PROGRAM


def _host_inputs(node_type, E, Wh, bh, Wc, bc):
    """Build per-core input maps (host side: sharding + index re-encoding)."""
    nt = np.asarray(node_type).astype(np.int64).reshape(B, M)
    E = np.asarray(E, dtype=np.float32)
    Wh = np.asarray(Wh, dtype=np.float32)
    bh = np.asarray(bh, dtype=np.float32)
    Wc = np.asarray(Wc, dtype=np.float32)
    bc = np.asarray(bc, dtype=np.float32)

    shared = {
        "e_bf": E.astype(np.float16),
        "e_t": np.ascontiguousarray(E.T),
        "wh_b": Wh.astype(np.float16),
        "bh": bh.reshape(H, 1),
        "wc": Wc,
        "bc": bc.reshape(NCLS, 1),
    }

    ncols = {d: np.arange(LVL_N[d]) for d in range(9)}
    in_maps = []
    for c in range(CORES):
        ntc = nt[c * TPC:(c + 1) * TPC]                  # [32, 1023]
        ohf = np.zeros((V, TOTAL_COLS), dtype=np.float32)
        # level types in tree-major local order, then permute to stored order
        lvl = {d: ntc[:, (1 << d) - 1:(2 << d) - 1].reshape(-1) for d in range(9)}
        leaves = ntc[:, 511:1023].reshape(-1)            # [16384]
        # level 8 bands: x8 + c8 (summed leaf-children one-hots)
        t8 = lvl[8][_ORD[8]]
        a = leaves[2 * _ORD[8]]
        b = leaves[2 * _ORD[8] + 1]
        for k in range(4):
            cols = np.arange(2048)
            ohf[t8[2048 * k + cols], OFF_B8[k] + cols] = 1.0
            np.add.at(ohf, (a[2048 * k + cols], OFF_B8[k] + 2048 + cols), 1.0)
            np.add.at(ohf, (b[2048 * k + cols], OFF_B8[k] + 2048 + cols), 1.0)
        # x7 band
        ohf[lvl[7][_ORD[7]], OFF_X7 + ncols[7]] = 1.0
        # x6..x0
        for d in range(6, -1, -1):
            ohf[lvl[d][_ORD[d]], OFF_REST + REST_OFF[d] + ncols[d]] = 1.0
        in_maps.append({"oh": ohf.astype(ml_dtypes.float8_e4m3), **shared})
    return in_maps


def kernel(node_type, parent_idx, depth, root_idx, E, Wh, bh, Wc, bc,
           _trace=False, _sim=False):
    from concourse.bass_utils import run_bass_kernel_spmd

    nc, out_name = _get_program()
    in_maps = _host_inputs(node_type, E, Wh, bh, Wc, bc)

    if _sim:
        from concourse.bass_interp import CoreSim
        outs = []
        for m in in_maps[:_sim if isinstance(_sim, int) and _sim > 1 else CORES]:
            sim = CoreSim(nc, trace=False)
            for k, v in m.items():
                sim.tensor(k)[:] = v
            sim.simulate(check_with_hw=False)
            outs.append(np.array(sim.tensor(out_name)))
        return np.concatenate(outs, axis=0).astype(np.float32)

    results = run_bass_kernel_spmd(
        nc, in_maps, core_ids=list(range(CORES)), trace=_trace,
    )
    out = np.concatenate([r[out_name] for r in results.results], axis=0)
    out = np.ascontiguousarray(out).astype(np.float32)
    if _trace:
        return out, results
    return out
